# revision 14
# baseline (speedup 1.0000x reference)
"""Trainium2 Bass kernel for a pre-LN transformer block (B=2,T=2048,D=1024,H=16).

Sharding: batch (2) x head-group (4 heads) -> 8 cores.
Per core: LN1 over its batch, QKV for its 4 heads (bf16 matmuls,
feature-major), causal attention in S^T layout (exp without max-subtraction,
masking via precomputed 0/1 mask tiles multiplied on DVE, softmax denominator
via an appended ones-row in the AV matmul), proj partials in token-major
layout with per-block bf16 ReduceScatter over the 4-core batch group issued
right after each block's heads (LN2 deferred so PE never stalls on the
collective), then token-parallel FFN (512 rows/core) with w_ff1 cached in
SBUF (prefetched under attention) and FFN1 split into token halves so the
first half covers the last RS chunk.
"""

import numpy as np
import ml_dtypes

import concourse.bass as bass
import concourse.bacc as bacc
import concourse.mybir as mybir
import concourse.tile as tile
from concourse.bass_utils import run_bass_kernel_spmd

F32 = mybir.dt.float32
BF16 = mybir.dt.bfloat16
I32 = mybir.dt.int32
ALU = mybir.AluOpType
AF = mybir.ActivationFunctionType
AX = mybir.AxisListType

BF16NP = ml_dtypes.bfloat16

B, T, D, H, HD = 2, 2048, 1024, 16, 64
F = 4 * D
NH = 4            # heads per core
TOK = 512         # own token rows per core (for FFN)
P = 128
EPS = 1e-5
N_CORES = 8

TT = T // P          # 16 token tiles per batch
DC = D // P          # 8 contraction chunks
FC = F // P          # 32 hidden chunks
NQB = T // 512       # 4 q blocks (also the RS chunk count)
OT = TOK // P        # 4 own-token tiles


def _rsqrt_newton(nc, stats, c15, x_ap, out_ap):
    """rsqrt of a [P,1] f32 tensor entirely on DVE (bit trick + 2 Newton
    steps); keeps the ACT engine free for the exp/square table set."""
    xi = stats.tile([P, 1], I32, name="xi", tag="xi")
    nc.vector.tensor_scalar(xi[:], x_ap.bitcast(I32), 1, None,
                            op0=ALU.arith_shift_right)
    y = stats.tile([P, 1], F32, name="y0n", tag="y0n")
    nc.vector.tensor_scalar(y[:].bitcast(I32), xi[:], 0x5F3759DF, -1,
                            op0=ALU.subtract, op1=ALU.mult)
    hx = stats.tile([P, 1], F32, name="hx", tag="hx")
    nc.vector.tensor_scalar_mul(hx[:], x_ap, 0.5)
    for it in range(2):
        y2 = stats.tile([P, 1], F32, name=f"y2{it}", tag="y2")
        nc.vector.tensor_tensor(y2[:], y[:], y[:], op=ALU.mult)
        tt = stats.tile([P, 1], F32, name=f"tt{it}", tag="tt")
        nc.vector.scalar_tensor_tensor(tt[:], y2[:], hx[:], c15[:],
                                       op0=ALU.mult, op1=ALU.subtract)
        yn = (stats.tile([P, 1], F32, name=f"yn{it}", tag="yn")
              if it < 1 else out_ap)
        dst = yn[:] if it < 1 else yn
        nc.vector.scalar_tensor_tensor(dst, y[:], -1.0, tt[:],
                                       op0=ALU.mult, op1=ALU.mult)
        y = yn if it < 1 else None


def _ln_tile(nc, stats, c15, xt_ap, out_tile_ap):
    """LayerNorm of one [P, D] token-major tile -> out tile (bf16).
    Uses the out tile as the scratch target of the ACT square pass."""
    ssum = stats.tile([P, 1], F32, name="ssum", tag="ssum")
    nc.vector.tensor_reduce(ssum[:], xt_ap, axis=AX.X, op=ALU.add)
    ssq = stats.tile([P, 1], F32, name="ssq", tag="ssq")
    nc.scalar.activation(out_tile_ap, xt_ap, AF.Square, accum_out=ssq[:])
    mu = stats.tile([P, 1], F32, name="mu", tag="mu")
    nc.vector.tensor_scalar_mul(mu[:], ssum[:], 1.0 / D)
    mu2 = stats.tile([P, 1], F32, name="mu2", tag="mu2")
    nc.vector.tensor_tensor(mu2[:], mu[:], mu[:], op=ALU.mult)
    var = stats.tile([P, 1], F32, name="var", tag="var")
    nc.vector.scalar_tensor_tensor(
        var[:], ssq[:], 1.0 / D, mu2[:], op0=ALU.mult, op1=ALU.subtract)
    vare = stats.tile([P, 1], F32, name="vare", tag="vare")
    nc.vector.tensor_scalar_add(vare[:], var[:], EPS)
    rstd = stats.tile([P, 1], F32, name="rstd", tag="rstd")
    _rsqrt_newton(nc, stats, c15, vare[:], rstd[:])
    nmr = stats.tile([P, 1], F32, name="nmr", tag="nmr")
    nc.vector.scalar_tensor_tensor(
        nmr[:], mu[:], -1.0, rstd[:], op0=ALU.mult, op1=ALU.mult)
    nc.vector.tensor_scalar(out_tile_ap, xt_ap, rstd[:], nmr[:],
                            op0=ALU.mult, op1=ALU.add)


def _transpose_tile(nc, tpps, identity, src_ap, dst_ap):
    """[P, D] token-major bf16 tile -> feature-major dst [P, DC, P] slices,
    batching 4 PE transposes per PSUM bank eviction (DVE)."""
    for jj in range(DC // 4):
        tp = tpps.tile([P, 4, P], BF16, name="tp", tag="tp")
        for j4 in range(4):
            j = 4 * jj + j4
            nc.tensor.transpose(tp[:, j4, :], src_ap[:, j * P:(j + 1) * P],
                                identity[:])
        nc.vector.tensor_copy(dst_ap[:, 4 * jj:4 * jj + 4, :], tp[:])


def build_nc(collective="chunked"):
    nc = bacc.Bacc("TRN2", target_bir_lowering=False, debug=False,
                   num_devices=N_CORES)

    x_b = nc.dram_tensor("x_b", [T, D], BF16, kind="ExternalInput")
    x_own = nc.dram_tensor("x_own", [TOK, D], F32, kind="ExternalInput")
    wq_s = nc.dram_tensor("wq_s", [D, NH * HD], BF16, kind="ExternalInput")
    wk_s = nc.dram_tensor("wk_s", [D, NH * HD], BF16, kind="ExternalInput")
    wv_s = nc.dram_tensor("wv_s", [D, NH * HD], BF16, kind="ExternalInput")
    w_proj_s = nc.dram_tensor("w_proj_s", [NH * HD, D], BF16,
                              kind="ExternalInput")
    w_ff1 = nc.dram_tensor("w_ff1", [D, F], BF16, kind="ExternalInput")
    b_ff1 = nc.dram_tensor("b_ff1", [F], F32, kind="ExternalInput")
    w_ff2 = nc.dram_tensor("w_ff2", [F, D], BF16, kind="ExternalInput")
    b_ff2 = nc.dram_tensor("b_ff2", [D], BF16, kind="ExternalInput")
    ident_h = nc.dram_tensor("ident_h", [P, P], BF16, kind="ExternalInput")
    ones_h = nc.dram_tensor("ones_h", [P, P], BF16, kind="ExternalInput")
    mask_h = nc.dram_tensor("mask_h", [P, 4 * 512], BF16,
                            kind="ExternalInput")
    out_own = nc.dram_tensor("out_own", [TOK, D], BF16,
                             kind="ExternalOutput")

    with tile.TileContext(nc) as tc:
        with (
            tc.tile_pool(name="const", bufs=1) as constp,
            tc.tile_pool(name="acts", bufs=1) as acts,
            tc.tile_pool(name="dram", bufs=1, space="DRAM") as dram,
        ):
            identity = constp.tile([P, P], BF16)
            nc.sync.dma_start(identity[:], ident_h[:])
            ones64 = constp.tile([1, HD], BF16)
            nc.sync.dma_start(ones64[:], ones_h[0:1, 0:HD])
            ones128 = constp.tile([1, P], BF16)
            nc.sync.dma_start(ones128[:], ones_h[0:1, :])
            c15 = constp.tile([P, 1], F32)
            nc.vector.memset(c15[:], 1.5)
            mask_sb = constp.tile([P, 4, 512], BF16)
            nc.sync.dma_start(mask_sb[:],
                              mask_h[:].rearrange("p (m c) -> p m c", m=4))
            bff2_sb = constp.tile([1, D], BF16)
            nc.sync.dma_start(bff2_sb[:], b_ff2[:])
            bff1_sb = constp.tile([P, FC], F32)
            nc.sync.dma_start(bff1_sb[:],
                              b_ff1[:].rearrange("(f p) -> p f", p=P))

            # tiles surviving across phases
            x2 = acts.tile([P, OT, D], F32)
            h2T = acts.tile([P, DC, TOK], BF16)
            aT = acts.tile([P, FC, TOK], BF16)

            y_bounce = [dram.tile([512, D], BF16, name=f"yb{i}")
                        for i in range(NQB)]
            rs_out = [dram.tile([P, D], BF16, name=f"rso{i}")
                      for i in range(NQB)]

            # ============ phase A: LN1+QKV, attention, proj+RS ============
            with (
                tc.tile_pool(name="stats", bufs=3) as stats,
                tc.tile_pool(name="xio2", bufs=2) as xio2,
            ):
              with tc.tile_pool(name="qkvacts", bufs=1) as qkvacts:
                qT = qkvacts.tile([P, 2, T], BF16)
                kT = qkvacts.tile([P, 2, T], BF16)
                v_sb = qkvacts.tile([P, TT, NH, HD + 1], BF16)
                oT = qkvacts.tile([P, 2, T], BF16)

                # --- LN1 + h^T + QKV, one 512-token block at a time ---
                with (
                    tc.tile_pool(name="wqkv", bufs=1) as wqkv,
                    tc.tile_pool(name="xio", bufs=2) as xio,
                    tc.tile_pool(name="hTb", bufs=2) as hTbp,
                    tc.tile_pool(name="mainps", bufs=2,
                                 space="PSUM") as mainps,
                    tc.tile_pool(name="tpps", bufs=2, space="PSUM") as tpps,
                ):
                    wq_sb = wqkv.tile([P, DC, NH * HD], BF16)
                    wk_sb = wqkv.tile([P, DC, NH * HD], BF16)
                    wv_sb = wqkv.tile([P, DC, NH * HD], BF16)

                    def _load_qkv_weights():
                        nc.sync.dma_start(
                            v_sb[:, :, :, HD:HD + 1],
                            ones_h[:, 0:TT * NH].rearrange(
                                "p (t h o) -> p t h o", t=TT, h=NH))
                        for w_sb, w_dram in ((wq_sb, wq_s), (wk_sb, wk_s),
                                             (wv_sb, wv_s)):
                            wr = w_dram[:].rearrange("(j p) m -> p j m", p=P)
                            for j in range(DC):
                                nc.sync.dma_start(w_sb[:, j, :],
                                                  wr[:, j, :])

                    for n in range(NQB):
                        hTb = hTbp.tile([P, DC, 512], BF16, name=f"hTb{n}",
                                        tag="hTb")
                        for rr in range(4):
                            r = 4 * n + rr
                            xt = xio.tile([P, D], BF16, name="xt", tag="xt")
                            nc.sync.dma_start(xt[:],
                                              x_b[r * P:(r + 1) * P, :])
                            ht = xio.tile([P, D], BF16, name="ht", tag="ht")
                            _ln_tile(nc, stats, c15, xt[:], ht[:])
                            _transpose_tile(
                                nc, tpps, identity, ht[:],
                                hTb[:, :, rr * P:(rr + 1) * P])
                        if n == 0:
                            _load_qkv_weights()
                        # q^T / k^T for this token block
                        for w_sb, dst in ((wq_sb, qT), (wk_sb, kT)):
                            for p in range(2):
                                ps = mainps.tile([P, 512], F32, name="qkps",
                                                 tag="qkps")
                                for j in range(DC):
                                    nc.tensor.matmul(
                                        ps[:], w_sb[:, j, p * P:(p + 1) * P],
                                        hTb[:, j, :],
                                        start=(j == 0), stop=(j == DC - 1))
                                nc.vector.tensor_copy(
                                    dst[:, p, n * 512:(n + 1) * 512], ps[:])
                        # v (token-major + ones col) for this block
                        for rr in range(4):
                            t = 4 * n + rr
                            ps = mainps.tile([P, 256], F32, name="vps",
                                             tag="vps")
                            for j in range(DC):
                                nc.tensor.matmul(
                                    ps[:], hTb[:, j, rr * P:(rr + 1) * P],
                                    wv_sb[:, j, :],
                                    start=(j == 0), stop=(j == DC - 1))
                            nc.vector.tensor_copy(
                                v_sb[:, t, :, 0:HD],
                                ps[:].rearrange("p (h s) -> p h s", h=NH))

                # --- attention (qb-outer) + per-block proj + RS chunk ---
                with (
                    tc.tile_pool(name="wpp", bufs=1) as wpp,
                    tc.tile_pool(name="attps", bufs=2, space="PSUM") as attps,
                    tc.tile_pool(name="avps", bufs=2, space="PSUM") as avps,
                        tc.tile_pool(name="pp", bufs=3) as pp,
                    tc.tile_pool(name="recp", bufs=2) as recp,
                    tc.tile_pool(name="pjsb", bufs=3) as pjsb,
                ):
                    wp_sb = wpp.tile([P, 2, D], BF16)
                    nc.sync.dma_start(
                        wp_sb[:],
                        w_proj_s[:].rearrange("(c p) m -> p c m", p=P))

                    def _post_block(qb):
                        # proj for this qb's 4 token tiles -> bounce -> RS
                        for tt4 in range(4):
                            t = 4 * qb + tt4
                            for n2 in range(2):
                                ns = slice(n2 * 512, (n2 + 1) * 512)
                                ps = attps.tile([P, 512], F32,
                                                name="pjps", tag="pj")
                                for c2 in range(2):
                                    nc.tensor.matmul(
                                        ps[:], oT[:, c2, t * P:(t + 1) * P],
                                        wp_sb[:, c2, ns],
                                        start=(c2 == 0), stop=(c2 == 1))
                                ysb = pjsb.tile([P, 512], BF16, name="ysb",
                                                tag="ysb")
                                nc.vector.tensor_copy(ysb[:], ps[:])
                                nc.sync.dma_start(
                                    y_bounce[qb][tt4 * P:(tt4 + 1) * P, ns],
                                    ysb[:])

                        # RS chunk qb: [512, D] summed -> [128, D] shard
                        if collective == "chunked":
                            nc.gpsimd.collective_compute(
                                "ReduceScatter", ALU.add,
                                replica_groups=[[0, 1, 2, 3], [4, 5, 6, 7]],
                                ins=[y_bounce[qb][:].opt()],
                                outs=[rs_out[qb][:].opt()],
                            )
                        elif collective == "single":
                            if qb == NQB - 1:
                                for c4 in range(NQB):
                                    nc.gpsimd.collective_compute(
                                        "ReduceScatter", ALU.add,
                                        replica_groups=[[0, 1, 2, 3],
                                                        [4, 5, 6, 7]],
                                        ins=[y_bounce[c4][:].opt()],
                                        outs=[rs_out[c4][:].opt()],
                                    )
                        else:
                            nc.gpsimd.dma_start(
                                rs_out[qb][:],
                                y_bounce[qb][P:2 * P, :])

                    # residual + LN2 + h2^T for own-token chunk cq
                    def _ln2_chunk(cq, tpps2):
                        # loads on the ACT DGE queue: the rs_sb load waits on
                        # the RS collective and must not block the SP queue
                        # (w_ff1 streaming) behind it
                        rs_sb = xio2.tile([P, D], BF16, name="rs_sb",
                                          tag="rs_sb")
                        nc.scalar.dma_start(rs_sb[:], rs_out[cq][:])
                        xo = xio2.tile([P, D], F32, name="xo", tag="xo")
                        nc.scalar.dma_start(
                            xo[:], x_own[cq * P:(cq + 1) * P, :])
                        nc.vector.tensor_copy(x2[:, cq, :], rs_sb[:])
                        nc.vector.tensor_tensor(x2[:, cq, :], x2[:, cq, :],
                                                xo[:], op=ALU.add)
                        h2t = xio2.tile([P, D], BF16, name="h2t",
                                        tag="h2t")
                        _ln_tile(nc, stats, c15, x2[:, cq, :], h2t[:])
                        _transpose_tile(nc, tpps2, identity, h2t[:],
                                        h2T[:, :, cq * P:(cq + 1) * P])

                    for qb in range(NQB):
                        qs = slice(qb * 512, (qb + 1) * 512)
                        for h in range(NH):
                            pr, s64 = h // 2, (h % 2) * HD
                            av = avps.tile([HD + 1, 512], F32, name="av",
                                           tag="av")
                            nkt = 4 * qb + 4

                            def _score_pair(k2):
                                sp = attps.tile([P, 2, 512], F32,
                                                name="sp", tag="sp")
                                for i in range(2):
                                    kt = 2 * k2 + i
                                    nc.tensor.matmul(
                                        sp[:, i, :],
                                        kT[s64:s64 + HD, pr,
                                           kt * P:(kt + 1) * P],
                                        qT[s64:s64 + HD, pr, qs],
                                        start=True, stop=True)
                                pt = pp.tile([P, 2, 512], BF16, name="pt",
                                             tag="pt")
                                nc.scalar.activation(
                                    pt[:], sp[:], AF.Exp,
                                    scale=float(HD) ** -0.5)
                                for i in range(2):
                                    kt = 2 * k2 + i
                                    if kt >= 4 * qb:
                                        nc.vector.tensor_tensor(
                                            pt[:, i, :], pt[:, i, :],
                                            mask_sb[:, kt - 4 * qb, :],
                                            op=ALU.mult)
                                return pt

                            def _av_pair(k2, pt):
                                for i in range(2):
                                    kt = 2 * k2 + i
                                    nc.tensor.matmul(
                                        av[:], v_sb[:, kt, h, :],
                                        pt[:, i, :], start=(kt == 0),
                                        stop=(kt == nkt - 1))

                            prev = None
                            for k2 in range(nkt // 2):
                                cur = _score_pair(k2)
                                if prev is not None:
                                    _av_pair(k2 - 1, prev)
                                prev = cur
                            _av_pair(nkt // 2 - 1, prev)
                            rec = recp.tile([1, 512], BF16, name="rec",
                                            tag="rec")
                            with nc.allow_low_precision(reason="bf16 recip"):
                                nc.vector.reciprocal(rec[:],
                                                     av[HD:HD + 1, :])
                            bc = attps.tile([HD, 512], F32, name="bc",
                                           tag="pj")
                            nc.tensor.matmul(bc[:], ones64[:], rec[:],
                                             start=True, stop=True)
                            bcs = pp.tile([HD, 512], F32, name="bcs",
                                          tag="bcs")
                            nc.vector.tensor_copy(bcs[:], bc[:])
                            with nc.allow_low_precision(reason="bf16 oT"):
                                nc.vector.tensor_tensor(
                                    oT[s64:s64 + HD, pr, qs], av[0:HD, :],
                                    bcs[:], op=ALU.mult)

                        _post_block(qb)

              # qkvacts closed; LN2 for the RS chunks that are done
              with tc.tile_pool(name="tpps2a", bufs=1,
                                space="PSUM") as tpps2a:
                  for cq in range(NQB - 1):
                      _ln2_chunk(cq, tpps2a)

              # ===== FFN1: cols 0..383 first (indep of RS chunk 3), w1
              # tiles stay resident, then cols 384..511 =====
              with (
                  tc.tile_pool(name="w1p", bufs=1) as w1p,
                  tc.tile_pool(name="ff1ps", bufs=2,
                               space="PSUM") as ff1ps,
              ):
                  w1tiles = []
                  for f in range(FC):
                      w1t = w1p.tile([P, DC, P], BF16, name=f"w1t{f}",
                                     tag=f"w1t{f}")
                      nc.sync.dma_start(
                          w1t[:],
                          w_ff1[:, f * P:(f + 1) * P].rearrange(
                              "(j p) m -> p j m", p=P))
                      w1tiles.append(w1t)
                      ps = ff1ps.tile([P, 384], F32, name="f1psa",
                                      tag="f1psa")
                      for j in range(DC):
                          nc.tensor.matmul(ps[:], w1t[:, j, :],
                                           h2T[:, j, 0:384],
                                           start=(j == 0),
                                           stop=(j == DC - 1))
                      with nc.allow_low_precision(reason="bf16 aT"):
                          nc.vector.tensor_scalar(
                              aT[:, f, 0:384], ps[:],
                              bff1_sb[:, f:f + 1], 0.0,
                              op0=ALU.add, op1=ALU.max)
                  with tc.tile_pool(name="tpps2b", bufs=1,
                                    space="PSUM") as tpps2b:
                      _ln2_chunk(NQB - 1, tpps2b)
                  for f in range(FC):
                      ps = ff1ps.tile([P, P], F32, name="f1psb",
                                      tag="f1psb")
                      for j in range(DC):
                          nc.tensor.matmul(ps[:], w1tiles[f][:, j, :],
                                           h2T[:, j, 384:512],
                                           start=(j == 0),
                                           stop=(j == DC - 1))
                      with nc.allow_low_precision(reason="bf16 aT"):
                          nc.vector.tensor_scalar(
                              aT[:, f, 384:512], ps[:],
                              bff1_sb[:, f:f + 1], 0.0,
                              op0=ALU.add, op1=ALU.max)

            # ============ FFN2 ============
            with (
                tc.tile_pool(name="w2p", bufs=4) as w2p,
                tc.tile_pool(name="ff2ps", bufs=1, space="PSUM") as ff2ps,
                tc.tile_pool(name="outp", bufs=4) as outp,
            ):
                pss = [ff2ps.tile([P, 512], F32, name=f"ff2ps_{i}",
                                  tag=f"ff2_{i}")
                       for i in range(OT * 2)]
                for f in range(FC):
                    w2t = w2p.tile([P, D], BF16, name="w2t", tag="w2t")
                    nc.sync.dma_start(w2t[:], w_ff2[f * P:(f + 1) * P, :])
                    for t in range(OT):
                        for n2 in range(2):
                            nc.tensor.matmul(
                                pss[t * 2 + n2][:],
                                aT[:, f, t * P:(t + 1) * P],
                                w2t[:, n2 * 512:(n2 + 1) * 512],
                                start=(f == 0), stop=False)
                for t in range(OT):
                    for n2 in range(2):
                        ns = slice(n2 * 512, (n2 + 1) * 512)
                        nc.tensor.matmul(pss[t * 2 + n2][:], ones128[:],
                                         bff2_sb[:, ns],
                                         start=False, stop=True)
                        ot = outp.tile([P, 512], BF16, name="ot",
                                       tag="ot")
                        with nc.allow_low_precision(reason="bf16 out"):
                            nc.vector.tensor_tensor(ot[:],
                                                    pss[t * 2 + n2][:],
                                                    x2[:, t, ns], op=ALU.add)
                        nc.sync.dma_start(
                            out_own[t * P:(t + 1) * P, ns], ot[:])

    nc.compile()
    return nc


_NC_CACHE = []
_last_in_maps = None


def _get_nc():
    if not _NC_CACHE:
        _NC_CACHE.append(build_nc())
    return _NC_CACHE[0]


def _make_masks():
    """mask[r, m, c] = 1.0 iff key row r of diagonal tile m is visible to
    query column c (keys: 128m + r <= c)."""
    r = np.arange(P)[:, None, None]
    m = np.arange(4)[None, :, None]
    c = np.arange(512)[None, None, :]
    return (c >= 128 * m + r).astype(BF16NP).reshape(P, 4 * 512)


def kernel(x, wq, wk, wv, w_proj, b_proj, w_ff1, b_ff1, w_ff2, b_ff2,
           ln1_g, ln1_b, ln2_g, ln2_b, **_ignored):
    x = np.asarray(x, np.float32)
    # fold LN gammas into the following projections (betas are zeros by spec)
    wq_f = np.asarray(wq, np.float32) * np.asarray(ln1_g, np.float32)[None, :, None]
    wk_f = np.asarray(wk, np.float32) * np.asarray(ln1_g, np.float32)[None, :, None]
    wv_f = np.asarray(wv, np.float32) * np.asarray(ln1_g, np.float32)[None, :, None]
    w_ff1_f = np.asarray(w_ff1, np.float32) * np.asarray(ln2_g, np.float32)[:, None]
    b_proj = np.asarray(b_proj, np.float32)
    masks = _make_masks()

    in_maps = []
    for c in range(N_CORES):
        b, g = c // 4, c % 4
        heads = slice(4 * g, 4 * g + 4)
        in_maps.append({
            "x_b": np.ascontiguousarray(x[b]).astype(BF16NP),
            # chunked RS: chunk qb hands core g batch rows 512*qb + 128*g,
            # i.e. this core owns row-tiles {tq : tq % 4 == g}
            "x_own": np.ascontiguousarray(
                x[b].reshape(TT, P, D)[g::4].reshape(TOK, D)) + b_proj,
            "wq_s": np.ascontiguousarray(
                wq_f[heads].transpose(1, 0, 2).reshape(D, NH * HD)).astype(BF16NP),
            "wk_s": np.ascontiguousarray(
                wk_f[heads].transpose(1, 0, 2).reshape(D, NH * HD)).astype(BF16NP),
            "wv_s": np.ascontiguousarray(
                wv_f[heads].transpose(1, 0, 2).reshape(D, NH * HD)).astype(BF16NP),
            "w_proj_s": np.ascontiguousarray(
                np.asarray(w_proj, np.float32)[256 * g:256 * g + 256]).astype(BF16NP),
            "w_ff1": w_ff1_f.astype(BF16NP),
            "b_ff1": np.asarray(b_ff1, np.float32),
            "w_ff2": np.asarray(w_ff2, np.float32).astype(BF16NP),
            "b_ff2": np.asarray(b_ff2, np.float32).astype(BF16NP),
            "ident_h": np.eye(P, dtype=np.float32).astype(BF16NP),
            "ones_h": np.ones((P, P), np.float32).astype(BF16NP),
            "mask_h": masks,
        })

    global _last_in_maps
    _last_in_maps = in_maps
    nc = _get_nc()
    res = run_bass_kernel_spmd(nc, in_maps, list(range(N_CORES)))

    out = np.empty((B, T, D), np.float32)
    ov = out.reshape(B, TT, P, D)
    for c in range(N_CORES):
        b, g = c // 4, c % 4
        ov[b, g::4] = res.results[c]["out_own"].reshape(OT, P, D)
    return out
